# revision 6
# baseline (speedup 1.0000x reference)
"""Trainium2 Bass kernel for nn_Attention_org_cross_85074712199395.

Reference computes two fully independent cross-attention branches:
  branch 0: Q(emb1;Wq)   x Kd,Vd(emb_alld;Wkd0,Wvd0) -> O1  via Wout
  branch 1: Qd(embd1;Wqd) x K,V  (emb_all; Wk0, Wv0 ) -> Od1 via Woutd
Sharding: 8 cores = 4 batches x 2 branches. Zero collectives.

v6: fused per-head pipeline, 3 O-sessions. v3 ran phase A (G + softmax
chains, DMA-bound, PE ~50% idle + HAM re-throttles) then phase B
(O = Ed^T M at the fp16 streaming roofline). v6 dissolves the split:
the O matmuls stream on the PE in 3 sessions as the per-head M blocks
land -- session0 = head 0 (starts ~20us, right after chain 0), session1
= head 1, session2 = heads 2+3 (their 4 chunks accumulate in PSUM, so
cross-session accumulation costs only two extra evac passes). G blocks
and chains for heads 1..3 are interleaved INTO the session streams so
the PE never idles on the eab DMA. Evacs use [128,1024] 2-bank PSUM
tiles -> one wide DVE/ACT op each (the ~400ns per-op fixed overhead is
what killed a 4-session variant: 256 narrow DVE ops = 167us of vector
time). Session0 copies alternate vector/scalar; session1/2 adds run on
vector into an SBUF fp16 ring; session2 adds write the out staging
tiles directly. Input DMA is split across both HWDGE rings (SP: eab
token-major stream + late edcm; ACT: weights + edcm heads 0-2).
PSUM = 8 banks: G, t2/pv shared, s, tb/pt shared, 2x2-bank m/O rotation.
"""

import sys
import types

import numpy as np

B, N, C, KV, H = 4, 4096, 1024, 1024, 4
Ch = C // H          # 256
EPS_ADJ = 1e-5 * KV  # InstanceNorm eps with the 1/sqrt(KV) score scale folded in


def _ensure_axon_hooks():
    """Inject antenv.axon_hooks (absent in this image) so trace=True works."""
    if "antenv.axon_hooks" in sys.modules:
        return
    try:
        import antenv  # noqa: F401
    except ImportError:
        return
    mod = types.ModuleType("antenv.axon_hooks")
    state = [None]
    mod.set_axon_ntff_profile_hook = lambda h: state.__setitem__(0, h)
    mod.get_axon_ntff_profile_hook = lambda: state[0]
    sys.modules["antenv.axon_hooks"] = mod
    try:
        from trn_agent_boot.trn_boot import _ntff_profile_via_ctypes

        mod.set_axon_ntff_profile_hook(
            _ntff_profile_via_ctypes("/opt/axon/libaxon_pjrt.so")
        )
    except Exception:
        pass


def build_nc(n_tokens=N):
    """Build + compile the per-core Bass program (SPMD-identical on all cores)."""
    import concourse.bass as bass
    import concourse.mybir as mybir
    import concourse.tile as tile
    from concourse import bacc
    from concourse.masks import make_identity

    f32 = mybir.dt.float32
    f16 = mybir.dt.float16
    Exp = mybir.ActivationFunctionType.Exp
    X = mybir.AxisListType.X
    np_ = n_tokens // 256
    n_tiles = n_tokens // 128

    nc = bacc.Bacc("TRN2", target_bir_lowering=False, debug=False, num_devices=8)

    # head-major token-major embeddings: [p, h, slice-pair, (sl01, ch)]
    eab_d = nc.dram_tensor("eab", [128, H * np_ * 4, 256], f16, kind="ExternalInput").ap()
    edcm_d = nc.dram_tensor("edcm", [128, 8, n_tokens], f16, kind="ExternalInput").ap()
    wq_d = nc.dram_tensor("wq", [128, 8, 256], f16, kind="ExternalInput").ap()
    wk_d = nc.dram_tensor("wk", [128, 2, 256], f16, kind="ExternalInput").ap()
    wv_d = nc.dram_tensor("wv", [128, 2, 256], f16, kind="ExternalInput").ap()
    wout_d = nc.dram_tensor("wout", [128, 8, 1024], f16, kind="ExternalInput").ap()
    out_d = nc.dram_tensor("out", [n_tokens, 1024], f16, kind="ExternalOutput").ap()

    from contextlib import ExitStack

    with tile.TileContext(nc) as tc:
        with ExitStack() as ctx:
            pool = lambda **kw: ctx.enter_context(tc.tile_pool(**kw))
            wpool = pool(name="weights", bufs=1)
            pt_pool = pool(name="pt", bufs=2)
            den_pool = pool(name="den", bufs=2)
            gsb_pool = pool(name="gsb", bufs=2)
            t2sb_pool = pool(name="t2sb", bufs=2)
            pvsb_pool = pool(name="pvsb", bufs=2)
            p_pool = pool(name="psb", bufs=2)
            stat_pool = pool(name="stat", bufs=8)
            scr_pool = pool(name="scr", bufs=1)
            msb_pool = pool(name="msb", bufs=1)
            s16_pool = pool(name="s16", bufs=2)
            e1_pool = pool(name="e1A", bufs=3)
            ekv_pool = pool(name="ekv", bufs=3)
            acc_pool = pool(name="accp", bufs=1)
            osb_pool = pool(name="osb", bufs=3)

            ident = wpool.tile([128, 128], f16)
            make_identity(nc, ident[:])
            ones_col = wpool.tile([128, 1], f32)
            nc.vector.memset(ones_col[:], 1.0)
            ones_row = wpool.tile([1, 128], f32)
            nc.vector.memset(ones_row[:], 1.0)
            warm_sb = wpool.tile([128, 128], f32)
            nc.vector.memset(warm_sb[:], 0.0)
            # preload the Sqrt and Exp activation tables off the critical path
            tdum = wpool.tile([1, 4], f32)
            nc.vector.memset(tdum[:1, 0:2], 1.0)
            nc.scalar.sqrt(tdum[:1, 2:3], tdum[:1, 0:1])
            nc.scalar.activation(tdum[:1, 3:4], tdum[:1, 1:2], Exp)

            wq = wpool.tile([128, 8, 256], f16)
            wk = wpool.tile([128, 2, 256], f16)
            wv = wpool.tile([128, 2, 256], f16)
            wout = wpool.tile([128, 8, 1024], f16)

            m_sb = msb_pool.tile([128, 8, 1024], f16)                  # M (K,o)
            scratch = scr_pool.tile([128, 512], f32)
            pv_sb = {}                                                 # h -> tile

            # warmup: get HAM to K=8/8 during the initial eab DMA window
            with tc.tile_pool(name="warm_ps", bufs=1, space="PSUM") as warm_pool:
                wps = warm_pool.tile([128, 512], f32)
                for w in range(12):
                    nc.tensor.matmul(wps[:, 0:128], warm_sb[:], warm_sb[:],
                                     start=(w == 0), stop=(w == 11))
                # verifier requires a reader for every written PSUM location
                nc.vector.tensor_copy(tdum[:1, 0:1], wps[:1, 0:1])

            # persistent PSUM pools: 1 + 1 + 1 + 1 + 2x2 = 8 banks
            g_ps = ctx.enter_context(tc.tile_pool(name="g_ps", bufs=1, space="PSUM"))
            t2pv_ps = ctx.enter_context(tc.tile_pool(name="t2pv", bufs=1, space="PSUM"))
            s_ps = ctx.enter_context(tc.tile_pool(name="s_ps", bufs=1, space="PSUM"))
            tbpt_ps = ctx.enter_context(tc.tile_pool(name="tbpt", bufs=1, space="PSUM"))
            mo_ps = ctx.enter_context(tc.tile_pool(name="mo_ps", bufs=2, space="PSUM"))

            inv = 1.0 / (256.0 * 256.0)

            # ---- ACT-ring DMA: weights + channel-major ed (heads 0..2) ----
            # ordered by first use; session0's prereqs (wout head0, edcm0)
            # come first so it can launch ~20us in
            ekv_tiles = [None] * H

            def load_ekv(h, eng):
                ekv_tiles[h] = ekv_pool.tile([128, 2, n_tokens], f16,
                                             name="ekvh", tag="ekvh")
                eng.dma_start(ekv_tiles[h][:], edcm_d[:, 2 * h:2 * h + 2, :])

            # issued from GPSIMD: descriptor-gen costs ~1-3us per dma_start
            # on the issuing engine's queue, which must not block the ACT
            # (sqrt/exp/m-evac) or SP (eab slab stream) rings
            nc.gpsimd.dma_start(wq[:], wq_d[:])
            nc.gpsimd.dma_start(wk[:], wk_d[:])
            nc.gpsimd.dma_start(wv[:], wv_d[:])
            nc.gpsimd.dma_start(wout[:, 0:2, :], wout_d[:, 0:2, :])
            load_ekv(0, nc.gpsimd)
            nc.gpsimd.dma_start(wout[:, 2:4, :], wout_d[:, 2:4, :])
            load_ekv(1, nc.gpsimd)
            nc.gpsimd.dma_start(wout[:, 4:8, :], wout_d[:, 4:8, :])
            load_ekv(2, nc.gpsimd)

            # ---- O accumulation ring: 34 x [128,1024] f16 ----
            acc_gens = {}                       # (sess, t) -> AP

            def acc_tile():
                return acc_pool.tile([128, 1024], f16, name="acc", tag="acc",
                                     bufs=34)

            SESS_HEADS = [(0,), (1,), (2, 3)]

            def emit_session_tile(sess, t):
                """O[t-block] partial = sum over session heads of Ed_h^T M_h."""
                heads = SESS_HEADS[sess]
                js = [2 * h + kc for h in heads for kc in range(2)]
                mp = mo_ps.tile([128, 1024], f32, name="mp", tag="mo")
                for oh in range(2):
                    for i, j in enumerate(js):
                        nc.tensor.matmul(
                            mp[:, oh * 512:(oh + 1) * 512],
                            ekv_tiles[j // 2][:, j % 2, t * 128:(t + 1) * 128],
                            m_sb[:, j, oh * 512:(oh + 1) * 512],
                            start=(i == 0), stop=(i == len(js) - 1))
                if sess == 0:
                    a = acc_tile()
                    if t % 2 == 0:
                        nc.vector.tensor_copy(a[:], mp[:])
                    else:
                        nc.scalar.copy(a[:], mp[:])
                    acc_gens[(0, t)] = a
                elif sess == 1:
                    a = acc_tile()
                    nc.vector.tensor_add(a[:], mp[:], acc_gens[(0, t)][:])
                    acc_gens[(1, t)] = a
                else:
                    ot = osb_pool.tile([128, 1024], f16, name="ot", tag="ot")
                    nc.vector.tensor_add(ot[:], mp[:], acc_gens[(1, t)][:])
                    r0 = t * 128
                    nc.sync.dma_start(out_d[r0:r0 + 128, :], ot[:])

            # ---- G: one 1MB slab (16 token slices) of head h's E1^T Ed ----
            g_state = {}
            nsl = np_ // 4

            def g_slab(h, sb):
                if sb == 0:
                    g_state[h] = g_ps.tile([128, 512], f32, name="gp", tag="gp")
                g_tile = g_state[h]
                j16 = (h * 4 + sb) * 16
                eab = e1_pool.tile([128, 16, 256], f16, name="eab", tag="eab")
                nc.sync.dma_start(eab[:], eab_d[:, j16:j16 + 16, :])
                for pr in range(4):
                    for half in range(2):
                        for cc in range(2):
                            nc.tensor.matmul(
                                g_tile[:, cc * 256:(cc + 1) * 256],
                                eab[:, 2 * pr + half, cc * 128:(cc + 1) * 128],
                                eab[:, 8 + 2 * pr + half, :],
                                start=(sb == 0 and pr == 0 and half == 0 and cc == 0),
                                stop=(sb == nsl - 1 and pr == 3 and half == 1 and cc == 1))

            def chain(h):
                """Per-head softmax/sandwich pipeline, emitted in 6 pieces."""
                g_tile = g_state[h]
                # --- P1a: G evac + T2 = (Wq G) as [k',oq] ---
                g_sb = gsb_pool.tile([128, 512], f16, name="gsb", tag="gsb")
                nc.vector.tensor_copy(g_sb[:, 0:256], g_tile[:, 0:256])
                nc.vector.tensor_copy(g_sb[:, 256:512], g_tile[:, 256:512])
                t2p = t2pv_ps.tile([128, 512], f32, name="t2p", tag="t2pv")
                for cc in range(2):
                    for kc in range(2):
                        nc.tensor.matmul(
                            t2p[:, kc * 256:(kc + 1) * 256],
                            g_sb[:, cc * 256 + kc * 128:cc * 256 + (kc + 1) * 128],
                            wq[:, 2 * h + cc, :],
                            start=(cc == 0 and kc == 0),
                            stop=(cc == 1 and kc == 1))
                yield
                # --- P1b: T2 evac + s = T2 Wkd^T ---
                t2_sb = t2sb_pool.tile([128, 512], f16, name="t2sb", tag="t2sb")
                nc.vector.tensor_copy(t2_sb[:, 0:256], t2p[:, 0:256])
                nc.vector.tensor_copy(t2_sb[:, 256:512], t2p[:, 256:512])
                s_t = s_ps.tile([128, 512], f32, name="s", tag="s")
                for kc in range(2):
                    for qc in range(2):
                        nc.tensor.matmul(
                            s_t[:, qc * 256:(qc + 1) * 256],
                            t2_sb[:, kc * 256 + qc * 128:kc * 256 + (qc + 1) * 128],
                            wk[:, kc, :],
                            start=(kc == 0 and qc == 0),
                            stop=(kc == 1 and qc == 1))
                yield
                # --- P1c: stats from an SBUF copy (DVE: one PSUM input only) ---
                s16 = s16_pool.tile([128, 512], f16, name="s16", tag="s16")
                nc.vector.tensor_copy(s16[:], s_t[:])
                stat_h = stat_pool.tile([128, 4], f32, name="st", tag="st")
                for cc in range(2):
                    s_ap = s16[:, cc * 256:(cc + 1) * 256]
                    nc.vector.reduce_sum(stat_h[:, cc:cc + 1], s_ap, axis=X)
                    nc.vector.tensor_mul(
                        scratch[:, cc * 256:(cc + 1) * 256], s_ap, s_ap)
                    nc.vector.reduce_sum(
                        stat_h[:, 2 + cc:3 + cc],
                        scratch[:, cc * 256:(cc + 1) * 256], axis=X)
                yield
                # --- P2: partition-sum, inorm scalars, broadcast ---
                tbh = tbpt_ps.tile([128, 8], f32, name="tbh", tag="tbpt")
                nc.tensor.matmul(tbh[:1, 0:4], ones_col[:], stat_h[:],
                                 start=True, stop=True)
                sch = stat_pool.tile([1, 16], f32, name="sc", tag="sc")
                nc.vector.tensor_copy(sch[:1, 0:4], tbh[:1, 0:4])
                nc.vector.tensor_add(sch[:1, 4:6], sch[:1, 0:4:2], sch[:1, 1:4:2])
                nc.vector.tensor_scalar_mul(sch[:1, 6:8], sch[:1, 4:6], inv)
                nc.vector.tensor_mul(sch[:1, 8:9], sch[:1, 6:7], sch[:1, 6:7])
                nc.vector.tensor_sub(sch[:1, 9:10], sch[:1, 7:8], sch[:1, 8:9])
                nc.vector.tensor_scalar_add(sch[:1, 10:11], sch[:1, 9:10], EPS_ADJ)
                nc.scalar.sqrt(sch[:1, 11:12], sch[:1, 10:11])
                nc.vector.reciprocal(sch[:1, 12:13], sch[:1, 11:12])
                nc.vector.tensor_mul(sch[:1, 13:14], sch[:1, 6:7], sch[:1, 12:13])
                nc.vector.tensor_scalar_mul(sch[:1, 14:15], sch[:1, 13:14], -1.0)
                nc.tensor.matmul(tbh[:, 4:6], ones_row[:], sch[:1, 12:15:2],
                                 start=True, stop=True)
                bch = stat_pool.tile([128, 2], f32, name="bc", tag="bc")
                nc.vector.tensor_copy(bch[:], tbh[:, 4:6])
                yield
                # --- P3: exp with accumulated denominators ---
                p = p_pool.tile([128, 512], f16, name="p", tag="p")
                den = den_pool.tile([128, 4], f32, name="den", tag="den")
                for cc in range(2):
                    nc.scalar.activation(
                        p[:, cc * 256:(cc + 1) * 256],
                        s_t[:, cc * 256:(cc + 1) * 256],
                        Exp, bias=bch[:, 1:2], scale=bch[:, 0:1],
                        accum_out=den[:, cc:cc + 1])
                    nc.vector.reciprocal(den[:, 2 + cc:3 + cc], den[:, cc:cc + 1])
                yield
                # --- P4: p^T and pv = (p Wvd)/den ---
                pts = []
                for kc in range(2):
                    ptp = tbpt_ps.tile([128, 256], f16, name="ptp", tag="tbpt")
                    for cc in range(2):
                        nc.tensor.transpose(
                            ptp[:, cc * 128:(cc + 1) * 128],
                            p[:, cc * 256 + kc * 128:cc * 256 + (kc + 1) * 128],
                            ident[:])
                    pt_sb = pt_pool.tile([128, 256], f16, name="ptsb", tag="ptsb")
                    nc.vector.tensor_copy(pt_sb[:], ptp[:])
                    pts.append(pt_sb)
                pvp = t2pv_ps.tile([128, 512], f32, name="pvp", tag="t2pv")
                for cc in range(2):
                    for kc in range(2):
                        nc.tensor.matmul(
                            pvp[:, cc * 256:(cc + 1) * 256],
                            pts[kc][:, cc * 128:(cc + 1) * 128],
                            wv[:, kc, :],
                            start=(cc == 0 and kc == 0),
                            stop=(cc == 1 and kc == 1))
                pv = pvsb_pool.tile([128, 512], f16, name="pv", tag="pv")
                for cc in range(2):
                    nc.vector.tensor_scalar_mul(
                        pv[:, cc * 256:(cc + 1) * 256],
                        pvp[:, cc * 256:(cc + 1) * 256],
                        den[:, 2 + cc:3 + cc])
                pv_sb[h] = pv
                yield

            def m_head(h):
                """M[(h,kk),o] = sum_c pv[c,kk] WoutT[(h,c),o] -> m_sb."""
                for kc2 in range(2):
                    mp = mo_ps.tile([128, 1024], f32, name="mhp", tag="mo")
                    for oh in range(2):
                        for cc in range(2):
                            nc.tensor.matmul(
                                mp[:, oh * 512:(oh + 1) * 512],
                                pv_sb[h][:, cc * 256 + kc2 * 128:cc * 256 + (kc2 + 1) * 128],
                                wout[:, 2 * h + cc, oh * 512:(oh + 1) * 512],
                                start=(cc == 0), stop=(cc == 1))
                    nc.scalar.copy(m_sb[:, 2 * h + kc2, :], mp[:])

            # ================= fused schedule =====================
            def run_session(sess, events):
                """Emit a session's tiles with chain/G events interleaved.

                events: {tile_index: [callable, ...]} run before that tile.
                """
                for t in range(n_tiles):
                    for fn in events.get(t, ()):
                        fn()
                    emit_session_tile(sess, t)
                for fn in events.get(n_tiles, ()):
                    fn()

            # head 0: DMA-bound startup; warmup keeps HAM at full clock
            for sb in range(nsl):
                g_slab(0, sb)
            c0 = chain(0)
            for _ in c0:
                pass
            m_head(0)

            # session0 + head 1's G/chain interleaved
            c1 = chain(1)
            run_session(0, {
                4: [lambda: g_slab(1, 0)], 6: [lambda: g_slab(1, 1)],
                8: [lambda: g_slab(1, 2)], 10: [lambda: g_slab(1, 3)],
                12: [lambda: next(c1)], 14: [lambda: next(c1)],
                16: [lambda: next(c1)], 18: [lambda: next(c1)],
                20: [lambda: next(c1)], 22: [lambda: next(c1)],
                26: [lambda: m_head(1)],
            })

            # session1 + heads 2,3 G/chain interleaved
            c2 = chain(2)
            c3 = chain(3)
            run_session(1, {
                0: [lambda: g_slab(2, 0)], 2: [lambda: g_slab(2, 1)],
                4: [lambda: g_slab(2, 2)], 6: [lambda: g_slab(2, 3)],
                8: [lambda: next(c2), lambda: load_ekv(3, nc.gpsimd)],
                9: [lambda: next(c2)],
                10: [lambda: next(c2)], 11: [lambda: next(c2)],
                12: [lambda: next(c2)], 14: [lambda: next(c2)],
                16: [lambda: m_head(2)],
                17: [lambda: g_slab(3, 0)], 19: [lambda: g_slab(3, 1)],
                21: [lambda: g_slab(3, 2)], 23: [lambda: g_slab(3, 3)],
                25: [lambda: next(c3)], 26: [lambda: next(c3)],
                27: [lambda: next(c3)], 28: [lambda: next(c3)],
                29: [lambda: next(c3)], 31: [lambda: next(c3)],
                n_tiles: [lambda: m_head(3)],
            })

            # session2: heads 2+3, writes out staging + DMA
            run_session(2, {})

    nc.compile()
    return nc


# ---------------- host-side data prep ----------------

def _prep_emb_hm(e):
    # [nt, 1024] -> [128, H*(nt//256)*2, 256]: head-major token slices;
    # A[p, (h, sp, sl01), cx] = e[(2*sp+sl01)*128+p, h*256+cx]
    nt = e.shape[0]
    a = e.reshape(nt // 256, 2, 128, 4, 256)          # [sp, sl01, p, h, cx]
    a = a.transpose(2, 3, 0, 1, 4).reshape(128, 8 * (nt // 256), 256)
    return np.ascontiguousarray(a.astype(np.float16))


def _prep_embT(e):
    # [nt, 1024] -> [128, 8, nt]: partition p, chunk cc -> channel cc*128+p
    return np.ascontiguousarray(
        e.T.reshape(8, 128, -1).transpose(1, 0, 2).astype(np.float16))


def _prep_wq(Wq):
    # [H, o, c] -> WqT [h, c, o] -> [128, (h,cc), 256]
    WqT = Wq.transpose(0, 2, 1)
    return np.ascontiguousarray(
        WqT.reshape(4, 2, 128, 256).transpose(2, 0, 1, 3).reshape(128, 8, 256)
        .astype(np.float16))


def _prep_wk(Wk):
    # [k, c] -> T [c, k] -> [128, cc, 256]  (WkdT chunks: rhs[p=k', j=ok])
    return np.ascontiguousarray(
        Wk.T.reshape(2, 128, 256).transpose(1, 0, 2).astype(np.float16))


def _prep_wv_native(Wv):
    # [kout, kin] native rows chunked: [128, kc, 256]  (rhs[p=k, j=kk])
    return np.ascontiguousarray(
        Wv.reshape(2, 128, 256).transpose(1, 0, 2).astype(np.float16))


def _prep_wout(Wo):
    # [o, C] with C=c*4+h -> Wo.T [C,o] -> head-major perm [h*256+c, o] -> chunks
    WoT = Wo.T.reshape(256, 4, 1024).transpose(1, 0, 2).reshape(1024, 1024)
    return np.ascontiguousarray(
        WoT.reshape(8, 128, 1024).transpose(1, 0, 2).astype(np.float16))


def make_in_maps(inputs):
    f = lambda x: np.asarray(x, dtype=np.float32)
    emb1, emb_all = f(inputs["emb1"]), f(inputs["emb_all"])
    embd1, emb_alld = f(inputs["embd1"]), f(inputs["emb_alld"])
    branch_w = [
        (_prep_wq(f(inputs["Wq"])), _prep_wk(f(inputs["Wkd0"])),
         _prep_wv_native(f(inputs["Wvd0"])), _prep_wout(f(inputs["Wout"]))),
        (_prep_wq(f(inputs["Wqd"])), _prep_wk(f(inputs["Wk0"])),
         _prep_wv_native(f(inputs["Wv0"])), _prep_wout(f(inputs["Woutd"]))),
    ]
    in_maps = []
    for core in range(8):
        b, br = core % 4, core // 4
        if br == 0:
            eq, ekv = emb1[b], emb_alld[b]
        else:
            eq, ekv = embd1[b], emb_all[b]
        wq, wk, wv, wo = branch_w[br]
        a = _prep_emb_hm(eq).reshape(128, 16, 8, 256)
        b = _prep_emb_hm(ekv).reshape(128, 16, 8, 256)
        eab = np.ascontiguousarray(
            np.concatenate([a, b], axis=2).reshape(128, 256, 256))
        in_maps.append({
            "eab": eab,
            "edcm": _prep_embT(ekv),
            "wq": wq, "wk": wk, "wv": wv, "wout": wo,
        })
    return in_maps


_NC_CACHE = {}


def get_nc(n_tokens=N):
    if n_tokens not in _NC_CACHE:
        _NC_CACHE[n_tokens] = build_nc(n_tokens)
    return _NC_CACHE[n_tokens]


def run_on_hw(in_maps, trace=False):
    _ensure_axon_hooks()
    from concourse.bass_utils import run_bass_kernel_spmd
    nc = get_nc()
    return run_bass_kernel_spmd(nc, in_maps, list(range(len(in_maps))), trace=trace)


def kernel(**inputs):
    res = run_on_hw(make_in_maps(inputs), trace=False)
    O1 = np.stack([np.float32(res.results[b]["out"]) for b in range(4)])
    Od1 = np.stack([np.float32(res.results[4 + b]["out"]) for b in range(4)])
    return O1, Od1


# revision 9
# speedup vs baseline: 1.0136x; 1.0136x over previous
"""Trainium2 Bass kernel for nn_Attention_org_cross_85074712199395.

Reference computes two fully independent cross-attention branches:
  branch 0: Q(emb1;Wq)   x Kd,Vd(emb_alld;Wkd0,Wvd0) -> O1  via Wout
  branch 1: Qd(embd1;Wqd) x K,V  (emb_all; Wk0, Wv0 ) -> Od1 via Woutd
Sharding: 8 cores = 4 batches x 2 branches. Zero collectives.

v6: fused per-head pipeline, 3 O-sessions. v3 ran phase A (G + softmax
chains, DMA-bound, PE ~50% idle + HAM re-throttles) then phase B
(O = Ed^T M at the fp16 streaming roofline). v6 dissolves the split:
the O matmuls stream on the PE in 3 sessions as the per-head M blocks
land -- session0 = head 0 (starts ~20us, right after chain 0), session1
= head 1, session2 = heads 2+3 (their 4 chunks accumulate in PSUM, so
cross-session accumulation costs only two extra evac passes). G blocks
and chains for heads 1..3 are interleaved INTO the session streams so
the PE never idles on the eab DMA. Evacs use [128,1024] 2-bank PSUM
tiles -> one wide DVE/ACT op each (the ~400ns per-op fixed overhead is
what killed a 4-session variant: 256 narrow DVE ops = 167us of vector
time). Session0 copies alternate vector/scalar; session1/2 adds run on
vector into an SBUF fp16 ring; session2 adds write the out staging
tiles directly. Input DMA is split across both HWDGE rings (SP: eab
token-major stream + late edcm; ACT: weights + edcm heads 0-2).
PSUM = 8 banks: G, t2/pv shared, s, tb/pt shared, 2x2-bank m/O rotation.
"""

import sys
import types

import numpy as np

B, N, C, KV, H = 4, 4096, 1024, 1024, 4
Ch = C // H          # 256
EPS_ADJ = 1e-5 * KV  # InstanceNorm eps with the 1/sqrt(KV) score scale folded in


def _ensure_axon_hooks():
    """Inject antenv.axon_hooks (absent in this image) so trace=True works."""
    if "antenv.axon_hooks" in sys.modules:
        return
    try:
        import antenv  # noqa: F401
    except ImportError:
        return
    mod = types.ModuleType("antenv.axon_hooks")
    state = [None]
    mod.set_axon_ntff_profile_hook = lambda h: state.__setitem__(0, h)
    mod.get_axon_ntff_profile_hook = lambda: state[0]
    sys.modules["antenv.axon_hooks"] = mod
    try:
        from trn_agent_boot.trn_boot import _ntff_profile_via_ctypes

        mod.set_axon_ntff_profile_hook(
            _ntff_profile_via_ctypes("/opt/axon/libaxon_pjrt.so")
        )
    except Exception:
        pass


def build_nc(n_tokens=N):
    """Build + compile the per-core Bass program (SPMD-identical on all cores)."""
    import concourse.bass as bass
    import concourse.mybir as mybir
    import concourse.tile as tile
    from concourse import bacc
    from concourse.masks import make_identity

    f32 = mybir.dt.float32
    f16 = mybir.dt.float16
    Exp = mybir.ActivationFunctionType.Exp
    X = mybir.AxisListType.X
    np_ = n_tokens // 256
    n_tiles = n_tokens // 128

    nc = bacc.Bacc("TRN2", target_bir_lowering=False, debug=False, num_devices=8)

    # head-major token-major embeddings: [p, h, slice-pair, (sl01, ch)]
    eab_d = nc.dram_tensor("eab", [128, H * np_ * 4, 256], f16, kind="ExternalInput").ap()
    edcm_d = nc.dram_tensor("edcm", [128, 8, n_tokens], f16, kind="ExternalInput").ap()
    wq_d = nc.dram_tensor("wq", [128, 8, 256], f16, kind="ExternalInput").ap()
    wk_d = nc.dram_tensor("wk", [128, 2, 256], f16, kind="ExternalInput").ap()
    wv_d = nc.dram_tensor("wv", [128, 2, 256], f16, kind="ExternalInput").ap()
    wout_d = nc.dram_tensor("wout", [128, 8, 1024], f16, kind="ExternalInput").ap()
    out_d = nc.dram_tensor("out", [n_tokens, 1024], f16, kind="ExternalOutput").ap()

    from contextlib import ExitStack

    with tile.TileContext(nc) as tc:
        with ExitStack() as ctx:
            pool = lambda **kw: ctx.enter_context(tc.tile_pool(**kw))
            wpool = pool(name="weights", bufs=1)
            pt_pool = pool(name="pt", bufs=2)
            den_pool = pool(name="den", bufs=2)
            gsb_pool = pool(name="gsb", bufs=2)
            t2sb_pool = pool(name="t2sb", bufs=2)
            pvsb_pool = pool(name="pvsb", bufs=2)
            p_pool = pool(name="psb", bufs=2)
            stat_pool = pool(name="stat", bufs=8)
            scr_pool = pool(name="scr", bufs=1)
            msb_pool = pool(name="msb", bufs=1)
            s16_pool = pool(name="s16", bufs=2)
            e1_pool = pool(name="e1A", bufs=3)
            ekv_pool = pool(name="ekv", bufs=3)
            acc_pool = pool(name="accp", bufs=1)
            osb_pool = pool(name="osb", bufs=3)

            ident = wpool.tile([128, 128], f16)
            make_identity(nc, ident[:])
            ones_col = wpool.tile([128, 1], f32)
            nc.vector.memset(ones_col[:], 1.0)
            ones_row = wpool.tile([1, 128], f32)
            nc.vector.memset(ones_row[:], 1.0)
            warm_sb = wpool.tile([128, 128], f32)
            nc.vector.memset(warm_sb[:], 0.0)
            # preload the Sqrt and Exp activation tables off the critical path
            tdum = wpool.tile([1, 4], f32)
            nc.vector.memset(tdum[:1, 0:2], 1.0)
            nc.scalar.sqrt(tdum[:1, 2:3], tdum[:1, 0:1])
            nc.scalar.activation(tdum[:1, 3:4], tdum[:1, 1:2], Exp)

            wq = wpool.tile([128, 8, 256], f16)
            wk = wpool.tile([128, 2, 256], f16)
            wv = wpool.tile([128, 2, 256], f16)
            wout = wpool.tile([128, 8, 1024], f16)

            m_sb = msb_pool.tile([128, 8, 1024], f16)                  # M (K,o)
            scratch = scr_pool.tile([128, 512], f32)
            pv_sb = {}                                                 # h -> tile

            # warmup: get HAM to K=8/8 during the initial eab DMA window
            with tc.tile_pool(name="warm_ps", bufs=1, space="PSUM") as warm_pool:
                wps = warm_pool.tile([128, 512], f32)
                for w in range(12):
                    nc.tensor.matmul(wps[:, 0:128], warm_sb[:], warm_sb[:],
                                     start=(w == 0), stop=(w == 11))
                # verifier requires a reader for every written PSUM location
                nc.vector.tensor_copy(tdum[:1, 0:1], wps[:1, 0:1])

            # persistent PSUM pools: 1 + 1 + 1 + 1 + 2x2 = 8 banks
            g_ps = ctx.enter_context(tc.tile_pool(name="g_ps", bufs=1, space="PSUM"))
            t2pv_ps = ctx.enter_context(tc.tile_pool(name="t2pv", bufs=1, space="PSUM"))
            s_ps = ctx.enter_context(tc.tile_pool(name="s_ps", bufs=1, space="PSUM"))
            tbpt_ps = ctx.enter_context(tc.tile_pool(name="tbpt", bufs=1, space="PSUM"))
            mo_ps = ctx.enter_context(tc.tile_pool(name="mo_ps", bufs=2, space="PSUM"))

            inv = 1.0 / (256.0 * 256.0)

            # ---- ACT-ring DMA: weights + channel-major ed (heads 0..2) ----
            # ordered by first use; session0's prereqs (wout head0, edcm0)
            # come first so it can launch ~20us in
            ekv_tiles = [None] * H

            def load_ekv(h, eng):
                ekv_tiles[h] = ekv_pool.tile([128, 2, n_tokens], f16,
                                             name="ekvh", tag="ekvh")
                eng.dma_start(ekv_tiles[h][:], edcm_d[:, 2 * h:2 * h + 2, :])

            # issued from GPSIMD: descriptor-gen costs ~1-3us per dma_start
            # on the issuing engine's queue, which must not block the ACT
            # (sqrt/exp/m-evac) or SP (eab slab stream) rings. Bulk loads
            # are paced with tiny gate-reads of m_sb rows so their fabric
            # traffic doesn't starve the latency-critical eab slab stream.
            gate_sb = wpool.tile([1, 8], f16)
            nc.gpsimd.dma_start(wq[:], wq_d[:])
            nc.gpsimd.dma_start(wk[:], wk_d[:])
            nc.gpsimd.dma_start(wv[:], wv_d[:])
            nc.gpsimd.dma_start(wout[:, 0:2, :], wout_d[:, 0:2, :])
            load_ekv(0, nc.gpsimd)

            def late_loads_1():
                # gate on m_head(0)'s first evac (~20us): wout/edcm for head 1
                nc.gpsimd.tensor_copy(gate_sb[:1, 0:2], m_sb[:1, 0, 0:2])
                nc.gpsimd.dma_start(wout[:, 2:4, :], wout_d[:, 2:4, :])
                load_ekv(1, nc.gpsimd)

            def late_loads_2():
                # gate on m_head(1)'s first evac (~45us): wout/edcm heads 2,3
                nc.gpsimd.tensor_copy(gate_sb[:1, 2:4], m_sb[:1, 2, 0:2])
                nc.gpsimd.dma_start(wout[:, 4:8, :], wout_d[:, 4:8, :])
                load_ekv(2, nc.gpsimd)

            # ---- O accumulation ring: 34 x [128,1024] f16 ----
            acc_gens = {}                       # (sess, t) -> AP

            def acc_tile():
                return acc_pool.tile([128, 1024], f16, name="acc", tag="acc",
                                     bufs=34)

            SESS_HEADS = [(0,), (1,), (2, 3)]

            def emit_session_tile(sess, t):
                """O[t-block] partial = sum over session heads of Ed_h^T M_h."""
                heads = SESS_HEADS[sess]
                js = [2 * h + kc for h in heads for kc in range(2)]
                mp = mo_ps.tile([128, 1024], f32, name="mp", tag="mo")
                for oh in range(2):
                    for i, j in enumerate(js):
                        nc.tensor.matmul(
                            mp[:, oh * 512:(oh + 1) * 512],
                            ekv_tiles[j // 2][:, j % 2, t * 128:(t + 1) * 128],
                            m_sb[:, j, oh * 512:(oh + 1) * 512],
                            start=(i == 0), stop=(i == len(js) - 1))
                if sess == 0:
                    a = acc_tile()
                    if t % 2 == 0:
                        nc.vector.tensor_copy(a[:], mp[:])
                    else:
                        nc.scalar.copy(a[:], mp[:])
                    acc_gens[(0, t)] = a
                elif sess == 1:
                    a = acc_tile()
                    nc.vector.tensor_add(a[:], mp[:], acc_gens[(0, t)][:])
                    acc_gens[(1, t)] = a
                else:
                    ot = osb_pool.tile([128, 1024], f16, name="ot", tag="ot")
                    nc.vector.tensor_add(ot[:], mp[:], acc_gens[(1, t)][:])
                    r0 = t * 128
                    nc.sync.dma_start(out_d[r0:r0 + 128, :], ot[:])

            # ---- G: one 1MB slab (16 token slices) of head h's E1^T Ed ----
            g_state = {}
            nsl = np_ // 4

            def g_slab(h, sb):
                if sb == 0:
                    g_state[h] = g_ps.tile([128, 512], f32, name="gp", tag="gp")
                g_tile = g_state[h]
                j16 = (h * 4 + sb) * 16
                eab = e1_pool.tile([128, 16, 256], f16, name="eab", tag="eab")
                nc.sync.dma_start(eab[:], eab_d[:, j16:j16 + 16, :])
                for pr in range(4):
                    for half in range(2):
                        for cc in range(2):
                            nc.tensor.matmul(
                                g_tile[:, cc * 256:(cc + 1) * 256],
                                eab[:, 2 * pr + half, cc * 128:(cc + 1) * 128],
                                eab[:, 8 + 2 * pr + half, :],
                                start=(sb == 0 and pr == 0 and half == 0 and cc == 0),
                                stop=(sb == nsl - 1 and pr == 3 and half == 1 and cc == 1))

            def chain(h):
                """Per-head softmax/sandwich pipeline, emitted in 6 pieces."""
                g_tile = g_state[h]
                # --- P1a: G evac + T2 = (Wq G) as [k',oq] ---
                g_sb = gsb_pool.tile([128, 512], f16, name="gsb", tag="gsb")
                nc.vector.tensor_copy(g_sb[:, 0:256], g_tile[:, 0:256])
                nc.vector.tensor_copy(g_sb[:, 256:512], g_tile[:, 256:512])
                t2p = t2pv_ps.tile([128, 512], f32, name="t2p", tag="t2pv")
                for cc in range(2):
                    for kc in range(2):
                        nc.tensor.matmul(
                            t2p[:, kc * 256:(kc + 1) * 256],
                            g_sb[:, cc * 256 + kc * 128:cc * 256 + (kc + 1) * 128],
                            wq[:, 2 * h + cc, :],
                            start=(cc == 0 and kc == 0),
                            stop=(cc == 1 and kc == 1))
                yield
                # --- P1b: T2 evac + s = T2 Wkd^T ---
                t2_sb = t2sb_pool.tile([128, 512], f16, name="t2sb", tag="t2sb")
                nc.vector.tensor_copy(t2_sb[:, 0:256], t2p[:, 0:256])
                nc.vector.tensor_copy(t2_sb[:, 256:512], t2p[:, 256:512])
                s_t = s_ps.tile([128, 512], f32, name="s", tag="s")
                for kc in range(2):
                    for qc in range(2):
                        nc.tensor.matmul(
                            s_t[:, qc * 256:(qc + 1) * 256],
                            t2_sb[:, kc * 256 + qc * 128:kc * 256 + (qc + 1) * 128],
                            wk[:, kc, :],
                            start=(kc == 0 and qc == 0),
                            stop=(kc == 1 and qc == 1))
                yield
                # --- P1c: stats from an SBUF copy (DVE: one PSUM input only) ---
                s16 = s16_pool.tile([128, 512], f16, name="s16", tag="s16")
                nc.vector.tensor_copy(s16[:], s_t[:])
                stat_h = stat_pool.tile([128, 4], f32, name="st", tag="st")
                for cc in range(2):
                    s_ap = s16[:, cc * 256:(cc + 1) * 256]
                    nc.vector.reduce_sum(stat_h[:, cc:cc + 1], s_ap, axis=X)
                    nc.vector.tensor_mul(
                        scratch[:, cc * 256:(cc + 1) * 256], s_ap, s_ap)
                    nc.vector.reduce_sum(
                        stat_h[:, 2 + cc:3 + cc],
                        scratch[:, cc * 256:(cc + 1) * 256], axis=X)
                yield
                # --- P2: partition-sum, inorm scalars, broadcast ---
                tbh = tbpt_ps.tile([128, 8], f32, name="tbh", tag="tbpt")
                nc.tensor.matmul(tbh[:1, 0:4], ones_col[:], stat_h[:],
                                 start=True, stop=True)
                sch = stat_pool.tile([1, 16], f32, name="sc", tag="sc")
                nc.vector.tensor_copy(sch[:1, 0:4], tbh[:1, 0:4])
                nc.vector.tensor_add(sch[:1, 4:6], sch[:1, 0:4:2], sch[:1, 1:4:2])
                nc.vector.tensor_scalar_mul(sch[:1, 6:8], sch[:1, 4:6], inv)
                nc.vector.tensor_mul(sch[:1, 8:9], sch[:1, 6:7], sch[:1, 6:7])
                nc.vector.tensor_sub(sch[:1, 9:10], sch[:1, 7:8], sch[:1, 8:9])
                nc.vector.tensor_scalar_add(sch[:1, 10:11], sch[:1, 9:10], EPS_ADJ)
                nc.scalar.sqrt(sch[:1, 11:12], sch[:1, 10:11])
                nc.vector.reciprocal(sch[:1, 12:13], sch[:1, 11:12])
                nc.vector.tensor_mul(sch[:1, 13:14], sch[:1, 6:7], sch[:1, 12:13])
                nc.vector.tensor_scalar_mul(sch[:1, 14:15], sch[:1, 13:14], -1.0)
                nc.tensor.matmul(tbh[:, 4:6], ones_row[:], sch[:1, 12:15:2],
                                 start=True, stop=True)
                bch = stat_pool.tile([128, 2], f32, name="bc", tag="bc")
                nc.vector.tensor_copy(bch[:], tbh[:, 4:6])
                yield
                # --- P3: exp with accumulated denominators ---
                p = p_pool.tile([128, 512], f16, name="p", tag="p")
                den = den_pool.tile([128, 4], f32, name="den", tag="den")
                for cc in range(2):
                    nc.scalar.activation(
                        p[:, cc * 256:(cc + 1) * 256],
                        s_t[:, cc * 256:(cc + 1) * 256],
                        Exp, bias=bch[:, 1:2], scale=bch[:, 0:1],
                        accum_out=den[:, cc:cc + 1])
                    nc.vector.reciprocal(den[:, 2 + cc:3 + cc], den[:, cc:cc + 1])
                yield
                # --- P4: p^T and pv = (p Wvd)/den ---
                pts = []
                for kc in range(2):
                    ptp = tbpt_ps.tile([128, 256], f16, name="ptp", tag="tbpt")
                    for cc in range(2):
                        nc.tensor.transpose(
                            ptp[:, cc * 128:(cc + 1) * 128],
                            p[:, cc * 256 + kc * 128:cc * 256 + (kc + 1) * 128],
                            ident[:])
                    pt_sb = pt_pool.tile([128, 256], f16, name="ptsb", tag="ptsb")
                    nc.vector.tensor_copy(pt_sb[:], ptp[:])
                    pts.append(pt_sb)
                pvp = t2pv_ps.tile([128, 512], f32, name="pvp", tag="t2pv")
                for cc in range(2):
                    for kc in range(2):
                        nc.tensor.matmul(
                            pvp[:, cc * 256:(cc + 1) * 256],
                            pts[kc][:, cc * 128:(cc + 1) * 128],
                            wv[:, kc, :],
                            start=(cc == 0 and kc == 0),
                            stop=(cc == 1 and kc == 1))
                pv = pvsb_pool.tile([128, 512], f16, name="pv", tag="pv")
                for cc in range(2):
                    nc.vector.tensor_scalar_mul(
                        pv[:, cc * 256:(cc + 1) * 256],
                        pvp[:, cc * 256:(cc + 1) * 256],
                        den[:, 2 + cc:3 + cc])
                pv_sb[h] = pv
                yield

            def m_head(h):
                """M[(h,kk),o] = sum_c pv[c,kk] WoutT[(h,c),o] -> m_sb."""
                for kc2 in range(2):
                    mp = mo_ps.tile([128, 1024], f32, name="mhp", tag="mo")
                    for oh in range(2):
                        for cc in range(2):
                            nc.tensor.matmul(
                                mp[:, oh * 512:(oh + 1) * 512],
                                pv_sb[h][:, cc * 256 + kc2 * 128:cc * 256 + (kc2 + 1) * 128],
                                wout[:, 2 * h + cc, oh * 512:(oh + 1) * 512],
                                start=(cc == 0), stop=(cc == 1))
                    nc.scalar.copy(m_sb[:, 2 * h + kc2, :], mp[:])

            # ================= fused schedule =====================
            def run_session(sess, events):
                """Emit a session's tiles with chain/G events interleaved.

                events: {tile_index: [callable, ...]} run before that tile.
                """
                for t in range(n_tiles):
                    for fn in events.get(t, ()):
                        fn()
                    emit_session_tile(sess, t)
                for fn in events.get(n_tiles, ()):
                    fn()

            # head 0: DMA-bound startup; warmup keeps HAM at full clock
            for sb in range(nsl):
                g_slab(0, sb)
            c0 = chain(0)
            for _ in c0:
                pass
            m_head(0)

            # session0 + head 1's G/chain interleaved
            c1 = chain(1)
            run_session(0, {
                0: [late_loads_1],
                8: [lambda: g_slab(1, 0)], 12: [lambda: g_slab(1, 1)],
                16: [lambda: g_slab(1, 2)], 20: [lambda: g_slab(1, 3)],
                21: [lambda: next(c1)], 22: [lambda: next(c1)],
                23: [lambda: next(c1)], 24: [lambda: next(c1)],
                25: [lambda: next(c1)], 27: [lambda: next(c1)],
                29: [lambda: m_head(1), late_loads_2],
            })

            # session1 + heads 2,3 G/chain interleaved
            c2 = chain(2)
            c3 = chain(3)
            run_session(1, {
                0: [lambda: g_slab(2, 0)], 2: [lambda: g_slab(2, 1)],
                4: [lambda: g_slab(2, 2)], 6: [lambda: g_slab(2, 3)],
                8: [lambda: next(c2), lambda: load_ekv(3, nc.gpsimd)],
                9: [lambda: next(c2)],
                10: [lambda: next(c2)], 11: [lambda: next(c2)],
                12: [lambda: next(c2)], 14: [lambda: next(c2)],
                16: [lambda: m_head(2)],
                17: [lambda: g_slab(3, 0)], 19: [lambda: g_slab(3, 1)],
                21: [lambda: g_slab(3, 2)], 23: [lambda: g_slab(3, 3)],
                24: [lambda: next(c3)], 25: [lambda: next(c3)],
                26: [lambda: next(c3)], 27: [lambda: next(c3)],
                28: [lambda: next(c3)], 29: [lambda: next(c3)],
                30: [lambda: m_head(3)],
            })

            # session2: heads 2+3, writes out staging + DMA
            run_session(2, {})

    nc.compile()
    return nc


# ---------------- host-side data prep ----------------

def _prep_emb_hm(e):
    # [nt, 1024] -> [128, H*(nt//256)*2, 256]: head-major token slices;
    # A[p, (h, sp, sl01), cx] = e[(2*sp+sl01)*128+p, h*256+cx]
    nt = e.shape[0]
    a = e.reshape(nt // 256, 2, 128, 4, 256)          # [sp, sl01, p, h, cx]
    a = a.transpose(2, 3, 0, 1, 4).reshape(128, 8 * (nt // 256), 256)
    return np.ascontiguousarray(a.astype(np.float16))


def _prep_embT(e):
    # [nt, 1024] -> [128, 8, nt]: partition p, chunk cc -> channel cc*128+p
    return np.ascontiguousarray(
        e.T.reshape(8, 128, -1).transpose(1, 0, 2).astype(np.float16))


def _prep_wq(Wq):
    # [H, o, c] -> WqT [h, c, o] -> [128, (h,cc), 256]
    WqT = Wq.transpose(0, 2, 1)
    return np.ascontiguousarray(
        WqT.reshape(4, 2, 128, 256).transpose(2, 0, 1, 3).reshape(128, 8, 256)
        .astype(np.float16))


def _prep_wk(Wk):
    # [k, c] -> T [c, k] -> [128, cc, 256]  (WkdT chunks: rhs[p=k', j=ok])
    return np.ascontiguousarray(
        Wk.T.reshape(2, 128, 256).transpose(1, 0, 2).astype(np.float16))


def _prep_wv_native(Wv):
    # [kout, kin] native rows chunked: [128, kc, 256]  (rhs[p=k, j=kk])
    return np.ascontiguousarray(
        Wv.reshape(2, 128, 256).transpose(1, 0, 2).astype(np.float16))


def _prep_wout(Wo):
    # [o, C] with C=c*4+h -> Wo.T [C,o] -> head-major perm [h*256+c, o] -> chunks
    WoT = Wo.T.reshape(256, 4, 1024).transpose(1, 0, 2).reshape(1024, 1024)
    return np.ascontiguousarray(
        WoT.reshape(8, 128, 1024).transpose(1, 0, 2).astype(np.float16))


def make_in_maps(inputs):
    f = lambda x: np.asarray(x, dtype=np.float32)
    emb1, emb_all = f(inputs["emb1"]), f(inputs["emb_all"])
    embd1, emb_alld = f(inputs["embd1"]), f(inputs["emb_alld"])
    branch_w = [
        (_prep_wq(f(inputs["Wq"])), _prep_wk(f(inputs["Wkd0"])),
         _prep_wv_native(f(inputs["Wvd0"])), _prep_wout(f(inputs["Wout"]))),
        (_prep_wq(f(inputs["Wqd"])), _prep_wk(f(inputs["Wk0"])),
         _prep_wv_native(f(inputs["Wv0"])), _prep_wout(f(inputs["Woutd"]))),
    ]
    in_maps = []
    for core in range(8):
        b, br = core % 4, core // 4
        if br == 0:
            eq, ekv = emb1[b], emb_alld[b]
        else:
            eq, ekv = embd1[b], emb_all[b]
        wq, wk, wv, wo = branch_w[br]
        a = _prep_emb_hm(eq).reshape(128, 16, 8, 256)
        b = _prep_emb_hm(ekv).reshape(128, 16, 8, 256)
        eab = np.ascontiguousarray(
            np.concatenate([a, b], axis=2).reshape(128, 256, 256))
        in_maps.append({
            "eab": eab,
            "edcm": _prep_embT(ekv),
            "wq": wq, "wk": wk, "wv": wv, "wout": wo,
        })
    return in_maps


_NC_CACHE = {}


def get_nc(n_tokens=N):
    if n_tokens not in _NC_CACHE:
        _NC_CACHE[n_tokens] = build_nc(n_tokens)
    return _NC_CACHE[n_tokens]


def run_on_hw(in_maps, trace=False):
    _ensure_axon_hooks()
    from concourse.bass_utils import run_bass_kernel_spmd
    nc = get_nc()
    return run_bass_kernel_spmd(nc, in_maps, list(range(len(in_maps))), trace=trace)


def kernel(**inputs):
    res = run_on_hw(make_in_maps(inputs), trace=False)
    O1 = np.stack([np.float32(res.results[b]["out"]) for b in range(4)])
    Od1 = np.stack([np.float32(res.results[4 + b]["out"]) for b in range(4)])
    return O1, Od1
